# revision 28
# baseline (speedup 1.0000x reference)
"""Batch-parallel attention kernel for 8 Trainium2 NeuronCores.

Problem: out[b,x,h] = sum_y softmax_y(sum_h' k[b,x,h']*q[b,y,h']) * v[b,y,h]
with q,k,v: [16, 2048, 128] fp32.  This is standard attention with the roles
of q and k swapped (queries = k rows, keys = q rows), no 1/sqrt(H) scale.

Sharding: batch dim (16) across 8 cores (pure data parallel), 2 batches per
core; flash-style x/y block tiling within a core.

Per-core algorithm (per batch, per x-half of 1024 score columns):
  Host supplies qT/kT = q/k transposed to [H, S] so H=128 sits on SBUF
  partitions (v stays natural).  For each y-block j (128 rows):
    sT_j[y, x]   = qT_j^T @ kT       (fp32r matmuls, N=512, PSUM)
    eT_j         = exp(sT_j - 30)    (ScalarE, PSUM -> SBUF, f32r out; the
                                      -30 shift widens overflow headroom and
                                      cancels exactly in the normalization)
    outT[h, x]  += v_j^T @ eT_j      (PSUM accumulate over all j)
    acc2(p)      = eT_2p + eT_2p+1   (DVE pair-sums)
    l[2, x]     += ones^T @ acc2     (softmax denominator; pair-sums halve
                                      the extra PE streaming; last pair is
                                      read directly from eT so the loop end
                                      never waits on a DVE add)
  No running-max subtraction is needed: scores are ~N(0, sqrt(128)) and the
  observed max ~84 stays far below the shifted overflow point (118.7).
  Tail per x-half: transpose l to [x,1] (K=1 matmuls), reciprocal on DVE,
  PE-transpose outT 128x128 blocks to [x, h], scale by 1/l during the
  PSUM->SBUF copy, DMA out in natural [S, H] layout.

Scheduling (the in-order engine queues make emission order = execution
order per engine):
  - MM1(j) is emitted one iteration ahead of MM2(j-1) so PE never idles
    waiting on exp(j) with useful MM1 work behind it.
  - The first two MM1/exp of the next (b, xh) are emitted inside the last
    two iterations of the current one, so ACT never drains at boundaries.
  - Each (b, xh)'s tail (l transpose / reciprocal / out transpose / scale /
    store) is deferred into the next loop's iterations 2 and 8, borrowing
    ps_s slots, so its DVE->PE dependency latency hides under real work.
  - A dummy-matmul chain + a dummy Exp at the start warm the PE HAM clock
    gate and preload the ACT table set while the first DMAs run.
PSUM budget (8 banks): 2x score slots (2 banks each) + outT accumulator
(2 banks) + l accumulator (2 banks).
"""
import os
import sys
import types
from contextlib import ExitStack

import numpy as np

import concourse.bass as bass
import concourse.tile as tile
from concourse import mybir
from concourse.bass_utils import run_bass_kernel_spmd
from concourse.masks import make_identity

F32 = mybir.dt.float32
F32R = mybir.dt.float32r
Act = mybir.ActivationFunctionType

B, S, H = 16, 2048, 128
NCORES = 8
BPC = B // NCORES  # batches per core
XH = 1024          # x-half width
NJ = S // 128      # y blocks


# ---------------------------------------------------------------------------
# Workaround: this walrus build rejects instructions carrying more than one
# semaphore wait ("Too many sync wait commands", seen on CTRL Drain and S3_LW
# Matmult).  Hoist all-but-one wait of every instruction onto wait-only
# EventSemaphore instructions on the same engine, inserted just before it.
_wsplit_counter = [0]


def _split_waits(nc, max_waits: int = 1):
    for func in nc.m.functions:
        for blk in func.blocks:
            insts = blk.instructions
            i = 0
            while i < len(insts):
                inst = insts[i]
                si = inst.sync_info
                waits = list(si.on_wait) if si is not None else []
                if len(waits) > max_waits:
                    keep = waits[-max_waits:]
                    hoist = waits[:-max_waits]
                    inst.sync_info = mybir.SyncInfo(
                        on_wait=keep, on_update=list(si.on_update)
                    )
                    new_insts = []
                    for w in hoist:
                        _wsplit_counter[0] += 1
                        ev = mybir.InstEventSemaphore(
                            name=f"WSPLIT-{_wsplit_counter[0]}", ins=[], outs=[]
                        )
                        ev.engine = inst.engine
                        ev.sync_info = mybir.SyncInfo(on_wait=[w], on_update=[])
                        new_insts.append(ev)
                    insts[i:i] = new_insts
                    i += len(new_insts)
                i += 1


# NTFF profiling shim: the axon .so supports NRT profiling but the antenv
# glue module is absent in this image; register it so trace=True works.
def _install_ntff_hook():
    if "antenv.axon_hooks" in sys.modules:
        return
    try:
        from trn_agent_boot.trn_boot import _ntff_profile_via_ctypes

        hook = _ntff_profile_via_ctypes("/opt/axon/libaxon_pjrt.so")
    except Exception:
        hook = None
    mod = types.ModuleType("antenv.axon_hooks")
    mod.get_axon_ntff_profile_hook = lambda: hook
    mod.set_axon_ntff_profile_hook = lambda h: None
    sys.modules["antenv.axon_hooks"] = mod


def _build():
    nc = bass.Bass("TRN2", target_bir_lowering=False, debug=False)
    qt = nc.dram_tensor("qt", [BPC, H, S], F32, kind="ExternalInput")
    kt = nc.dram_tensor("kt", [BPC, H, S], F32, kind="ExternalInput")
    v = nc.dram_tensor("v", [BPC, S, H], F32, kind="ExternalInput")
    out = nc.dram_tensor("out", [BPC, S, H], F32, kind="ExternalOutput")

    with tile.TileContext(nc) as tc, ExitStack() as ctx:
        consts = ctx.enter_context(tc.tile_pool(name="consts", bufs=1))
        raw = ctx.enter_context(tc.tile_pool(name="raw", bufs=4))
        qkv = ctx.enter_context(tc.tile_pool(name="qkv", bufs=2))
        et_pool = ctx.enter_context(tc.tile_pool(name="et", bufs=8))
        acc_pool = ctx.enter_context(tc.tile_pool(name="acc", bufs=5))
        sb_small = ctx.enter_context(tc.tile_pool(name="sb_small", bufs=2))
        outs = ctx.enter_context(tc.tile_pool(name="outs", bufs=2))
        ps_s = ctx.enter_context(tc.tile_pool(name="ps_s", bufs=2, space="PSUM"))
        ps_o = ctx.enter_context(tc.tile_pool(name="ps_o", bufs=1, space="PSUM"))
        ps_l = ctx.enter_context(tc.tile_pool(name="ps_l", bufs=1, space="PSUM"))

        ident = consts.tile([128, 128], F32)
        make_identity(nc, ident[:])
        # touch Exp first thing so the ACT table set loads under the DMAs
        warm = consts.tile([128, 2], F32)
        nc.vector.memset(warm[:], 0.0)
        nc.scalar.activation(warm[:], warm[:], Act.Exp)
        ones_f = consts.tile([128, 2], F32)
        nc.vector.memset(ones_f[:], 1.0)
        exp_bias = consts.tile([128, 1], F32)
        nc.vector.memset(exp_bias[:], -30.0)
        ones_r = consts.tile([128, 2], F32R)
        nc.vector.tensor_copy(ones_r[:], ones_f[:])
        # dummy matmul chain: keeps the PE busy during the initial DMAs so
        # the HAM clock-gate is at full rate when real matmuls arrive
        warm_z = consts.tile([128, 512], F32, tag="wz")
        nc.vector.memset(warm_z[:], 0.0)
        warm_r = consts.tile([128, 512], F32R)
        nc.vector.tensor_copy(warm_r[:], warm_z[:])
        ps_junk = ps_s.tile([128, 512], F32, tag="ps_s")
        for _ in range(10):
            nc.tensor.matmul(
                ps_junk[:], warm_r[:, 0:128], warm_r[:], start=True, stop=True
            )
        junk_sb = consts.tile([128, 2], F32, tag="wjunk")
        nc.vector.tensor_copy(junk_sb[:], ps_junk[:, 0:2])

        def emit_loads(b, fine):
            # load + round to f32r, chunked so compute starts early.
            # First batch uses finer leading chunks to cut the startup
            # serial path (DMA+cast before the first matmul).
            qr = qkv.tile([128, S], F32R, tag="qr")
            kr = qkv.tile([128, S], F32R, tag="kr")
            vr = qkv.tile([128, S], F32R, tag="vr")

            def load_k(lo, n):
                t = raw.tile([128, n], F32, tag="raw")
                nc.sync.dma_start(t[:], kt.ap()[b][:, bass.ds(lo, n)])
                nc.vector.tensor_copy(kr[:, bass.ds(lo, n)], t[:])

            def load_q(lo, n):
                t = raw.tile([128, n], F32, tag="raw")
                nc.sync.dma_start(t[:], qt.ap()[b][:, bass.ds(lo, n)])
                nc.vector.tensor_copy(qr[:, bass.ds(lo, n)], t[:])

            def load_v(lo, n):
                # v[b] rows [lo, lo+n) presented as [128p, (j 128h)]
                t = raw.tile([128, n], F32, tag="raw")
                v_chunk = bass.AP(
                    tensor=v,
                    offset=b * S * H + lo * H,
                    ap=[[H, 128], [128 * H, n // 128], [1, H]],
                )
                nc.sync.dma_start(t[:], v_chunk)
                nc.vector.tensor_copy(vr[:, bass.ds(lo, n)], t[:])

            if fine:
                load_k(0, 512)
                load_q(0, 256)
                load_v(0, 256)
                load_k(512, 512)
                load_q(256, 768)
                load_v(256, 768)
                load_k(1024, 1024)
                load_q(1024, 1024)
                load_v(1024, 1024)
            else:
                for hc in range(2):
                    load_k(hc * XH, XH)
                    load_q(hc * XH, XH)
                    load_v(hc * XH, XH)
            return qr, kr, vr

        qkv_b = {0: emit_loads(0, fine=True)}

        # Tail work for iteration (b, xh) is deferred into the NEXT
        # iteration's j-loop: the in-order PE queue then has the next
        # x-half's MM1s in front of the tail's small matmuls/transposes, so
        # the tail's DVE->PE dependency latency hides under real work.
        # part0 (emitted at loop end): evacuate po, then pl, on DVE.
        # part1 (next loop, it==3): l K=1 transposes + out transposes 0-3
        #   into one stolen ps_s slot; reciprocal; scale+copy 0-3.
        # part2 (next loop, it==9): out transposes 4-7; scale+copy; DMA out.
        def make_tail(b, xh, po, pl):
            st = {}

            def part0(l_first=False):
                if l_first:
                    l_sb = sb_small.tile([1, XH], F32R, tag="l_sb")
                    nc.vector.tensor_copy(l_sb[:], pl[0:1, :])
                outu = outs.tile([128, XH], F32, tag="outu")
                nc.vector.tensor_copy(outu[:], po[:])
                if not l_first:
                    l_sb = sb_small.tile([1, XH], F32R, tag="l_sb")
                    nc.vector.tensor_copy(l_sb[:], pl[0:1, :])
                out_sb = outs.tile([128, XH], F32, tag="out_sb")
                st.update(outu=outu, l_sb=l_sb, out_sb=out_sb)

            def part1():
                # one ps_s slot: cols 0-511 = transposes 0-3 (bank 0),
                # cols 512-527 = transposed l columns (bank 1).
                steal1 = ps_s.tile([128, 528], F32, tag="ps_s")
                for t in range(8):
                    nc.tensor.matmul(
                        steal1[:, 512 + 2 * t : 512 + 2 * t + 2],
                        st["l_sb"][0:1, bass.ts(t, 128)],
                        ones_r[0:1, 0:2],
                        start=True,
                        stop=True,
                    )
                for t in range(4):
                    nc.tensor.transpose(
                        steal1[:, bass.ts(t, 128)],
                        st["outu"][:, bass.ts(t, 128)],
                        ident[:],
                    )
                rl = sb_small.tile([128, 16], F32, tag="rl")
                nc.vector.reciprocal(rl[:], steal1[:, 512:528])
                for t in range(4):
                    nc.vector.tensor_scalar_mul(
                        st["out_sb"][:, bass.ts(t, 128)],
                        steal1[:, bass.ts(t, 128)],
                        rl[:, 2 * t : 2 * t + 1],
                    )
                st["rl"] = rl

            def part2():
                steal2 = ps_s.tile([128, 512], F32, tag="ps_s")
                for t in range(4):
                    nc.tensor.transpose(
                        steal2[:, bass.ts(t, 128)],
                        st["outu"][:, bass.ts(4 + t, 128)],
                        ident[:],
                    )
                for t in range(4):
                    nc.vector.tensor_scalar_mul(
                        st["out_sb"][:, bass.ts(4 + t, 128)],
                        steal2[:, bass.ts(t, 128)],
                        st["rl"][:, 2 * (4 + t) : 2 * (4 + t) + 1],
                    )
                # out[b] rows [xh*1024,...) as [128p, (8t 128h)], row=t*128+p
                out_view = bass.AP(
                    tensor=out,
                    offset=b * S * H + xh * 8 * 128 * H,
                    ap=[[H, 128], [128 * H, 8], [1, H]],
                )
                nc.sync.dma_start(out_view, st["out_sb"][:])

            return part0, part1, part2

        pending = None  # (part1, part2) of the previous (b, xh)

        # Software-pipelined emission: MM1(j) goes one iteration ahead of
        # MM2(j) so the in-order PE queue never waits on exp(j) with useful
        # MM1 work behind it, and the first two MM1/exp of the NEXT (b, xh)
        # are emitted inside the last two iterations of the current one so
        # the ACT exp chain never drains across loop boundaries.  The
        # softmax denominator is computed from DVE pair-sums
        # acc2(p) = eT(2p) + eT(2p+1), halving the PE ones-matmul streaming
        # cost, except the last pair which is read directly from eT so the
        # loop end does not wait on a DVE add.
        def emit_mm1_exp(qr, kr, xh, it, ets):
            pss = ps_s.tile([128, XH], F32, tag="ps_s")
            qj = qr[:, bass.ts(it, 128)]
            for c in range(2):
                nc.tensor.matmul(
                    pss[:, bass.ts(c, 512)],
                    qj,
                    kr[:, bass.ds(xh * XH + c * 512, 512)],
                    start=True,
                    stop=True,
                )
            et = et_pool.tile([128, XH], F32R, tag="et")
            ets[it] = et
            # bias -30 shifts the exp range: overflow now needs a score
            # > 118 instead of 88.7; the shift cancels exactly in the
            # softmax normalization (both numerator and l scale by e^-30)
            nc.scalar.activation(et[:], pss[:], Act.Exp, bias=exp_bias[:])

        seq = [(b, xh) for b in range(BPC) for xh in range(2)]
        heads = {}  # idx -> ets dict with pre-emitted iterations
        for idx, (b, xh) in enumerate(seq):
            qr, kr, vr = qkv_b[b]
            po = ps_o.tile([128, XH], F32)
            pl = ps_l.tile([2, XH], F32)
            ets = heads.pop(idx, {})
            accs = {}
            for it in range(NJ + 2):
                if it in (NJ, NJ + 1) and idx + 1 < len(seq):
                    # head of the next (b, xh): keep PE and ACT primed
                    nb, nxh = seq[idx + 1]
                    nqr, nkr, _ = qkv_b[nb]
                    h = heads.setdefault(idx + 1, {})
                    emit_mm1_exp(nqr, nkr, nxh, it - NJ, h)
                if it < NJ and it not in ets:
                    emit_mm1_exp(qr, kr, xh, it, ets)
                jj = it - 1
                if 0 <= jj < NJ:
                    vj = vr[:, bass.ts(jj, 128)]
                    for c in range(2):
                        nc.tensor.matmul(
                            po[:, bass.ts(c, 512)],
                            vj,
                            ets[jj][:, bass.ts(c, 512)],
                            start=(jj == 0),
                            stop=(jj == NJ - 1),
                        )
                # tail hook sits between MM2 and the pair-adds so the
                # tail's reciprocal/muls aren't queued behind a DVE add
                if pending is not None and it == 2:
                    pending[0]()
                if pending is not None and it == 8:
                    pending[1]()
                    pending = None
                if 0 <= jj < NJ:
                    if jj % 2 == 1 and jj < NJ - 2:
                        p = jj // 2
                        acc2 = acc_pool.tile([128, XH], F32R, tag="acc2")
                        accs[p] = acc2
                        nc.vector.tensor_add(
                            acc2[:], ets.pop(jj - 1)[:], ets.pop(jj)[:]
                        )
                    if jj == NJ - 1:
                        # last pair direct from eT: no DVE add on the
                        # loop-end critical path
                        last_two = [ets.pop(NJ - 2), ets.pop(NJ - 1)]
                        for srct in last_two:
                            for c in range(2):
                                nc.tensor.matmul(
                                    pl[0:2, bass.ts(c, 512)],
                                    ones_r[:],
                                    srct[:, bass.ts(c, 512)],
                                    start=False,
                                    stop=False,
                                )
                if it >= 5 and (it - 5) % 2 == 0 and (it - 5) // 2 < NJ // 2 - 1:
                    p = (it - 5) // 2
                    acc2 = accs.pop(p)
                    for c in range(2):
                        nc.tensor.matmul(
                            pl[0:2, bass.ts(c, 512)],
                            ones_r[:],
                            acc2[:, bass.ts(c, 512)],
                            start=(p == 0),
                            stop=(p == NJ // 2 - 2 and c == 1),
                        )
                if idx == 0 and it == 6 and BPC > 1:
                    # prefetch next batch; spread so the DVE casts don't
                    # pile up in one window
                    qkv_b[1] = emit_loads(1, fine=False)

            part0, part1, part2 = make_tail(b, xh, po, pl)
            part0(l_first=(idx == len(seq) - 1))
            pending = (part1, part2)

        # tail of the final (b, xh)
        pending[0]()
        pending[1]()

    _split_waits(nc)
    return nc


_NC_CACHE = None


def _get_nc():
    global _NC_CACHE
    if _NC_CACHE is None:
        _NC_CACHE = _build()
    return _NC_CACHE


def kernel(q: np.ndarray, k: np.ndarray, v: np.ndarray) -> np.ndarray:
    q = np.asarray(q, dtype=np.float32)
    k = np.asarray(k, dtype=np.float32)
    v = np.asarray(v, dtype=np.float32)
    qT = np.ascontiguousarray(q.transpose(0, 2, 1))  # [B, H, S]
    kT = np.ascontiguousarray(k.transpose(0, 2, 1))

    nc = _get_nc()
    in_maps = []
    for c in range(NCORES):
        sl = slice(BPC * c, BPC * (c + 1))
        in_maps.append(
            {
                "qt": np.ascontiguousarray(qT[sl]),
                "kt": np.ascontiguousarray(kT[sl]),
                "v": np.ascontiguousarray(v[sl]),
            }
        )

    trace = bool(int(os.environ.get("ATTN_KERNEL_TRACE", "0")))
    kwargs = {}
    if trace:
        _install_ntff_hook()
        kwargs["trace"] = True
        tmpdir = os.environ.get("ATTN_KERNEL_TRACE_DIR")
        if tmpdir:
            kwargs["tmpdir"] = tmpdir
    try:
        res = run_bass_kernel_spmd(
            nc, in_maps, core_ids=list(range(NCORES)), **kwargs
        )
    except Exception:
        # transient NRT/device hiccups have been observed once; retry
        res = run_bass_kernel_spmd(
            nc, in_maps, core_ids=list(range(NCORES)), **kwargs
        )
    if trace:
        kernel.last_results = res
    out = np.concatenate([res.results[c]["out"] for c in range(NCORES)], axis=0)
    return out.astype(np.float32)


# revision 29
# speedup vs baseline: 1.0276x; 1.0276x over previous
"""Batch-parallel attention kernel for 8 Trainium2 NeuronCores.

Problem: out[b,x,h] = sum_y softmax_y(sum_h' k[b,x,h']*q[b,y,h']) * v[b,y,h]
with q,k,v: [16, 2048, 128] fp32.  This is standard attention with the roles
of q and k swapped (queries = k rows, keys = q rows), no 1/sqrt(H) scale.

Sharding: batch dim (16) across 8 cores (pure data parallel), 2 batches per
core; flash-style x/y block tiling within a core.

Per-core algorithm (per batch, per x-half of 1024 score columns):
  Host supplies qT/kT = q/k transposed to [H, S] so H=128 sits on SBUF
  partitions (v stays natural).  For each y-block j (128 rows):
    sT_j[y, x]   = qT_j^T @ kT       (fp32r matmuls, N=512, PSUM)
    eT_j         = exp(sT_j - 30)    (ScalarE, PSUM -> SBUF, f32r out; the
                                      -30 shift widens overflow headroom and
                                      cancels exactly in the normalization)
    outT[h, x]  += v_j^T @ eT_j      (PSUM accumulate over all j)
    acc2(p)      = eT_2p + eT_2p+1   (DVE pair-sums)
    l[2, x]     += ones^T @ acc2     (softmax denominator; pair-sums halve
                                      the extra PE streaming; last pair is
                                      read directly from eT so the loop end
                                      never waits on a DVE add)
  No running-max subtraction is needed: scores are ~N(0, sqrt(128)) and the
  observed max ~84 stays far below the shifted overflow point (118.7).
  Tail per x-half: transpose l to [x,1] (K=1 matmuls), reciprocal on DVE,
  PE-transpose outT 128x128 blocks to [x, h], scale by 1/l during the
  PSUM->SBUF copy, DMA out in natural [S, H] layout.

Scheduling (the in-order engine queues make emission order = execution
order per engine):
  - MM1(j) is emitted one iteration ahead of MM2(j-1) so PE never idles
    waiting on exp(j) with useful MM1 work behind it.
  - The first two MM1/exp of the next (b, xh) are emitted inside the last
    two iterations of the current one, so ACT never drains at boundaries.
  - Each (b, xh)'s tail (l transpose / reciprocal / out transpose / scale /
    store) is deferred into the next loop's iterations 2 and 8, borrowing
    ps_s slots, so its DVE->PE dependency latency hides under real work.
  - A dummy-matmul chain + a dummy Exp at the start warm the PE HAM clock
    gate and preload the ACT table set while the first DMAs run.
PSUM budget (8 banks): 2x score slots (2 banks each) + outT accumulator
(2 banks) + l accumulator (2 banks).
"""
import os
import sys
import types
from contextlib import ExitStack

import numpy as np

import concourse.bass as bass
import concourse.tile as tile
from concourse import mybir
from concourse.bass_utils import run_bass_kernel_spmd
from concourse.masks import make_identity

F32 = mybir.dt.float32
F32R = mybir.dt.float32r
Act = mybir.ActivationFunctionType

B, S, H = 16, 2048, 128
NCORES = 8
BPC = B // NCORES  # batches per core
XH = 1024          # x-half width
NJ = S // 128      # y blocks


# ---------------------------------------------------------------------------
# Workaround: this walrus build rejects instructions carrying more than one
# semaphore wait ("Too many sync wait commands", seen on CTRL Drain and S3_LW
# Matmult).  Hoist all-but-one wait of every instruction onto wait-only
# EventSemaphore instructions on the same engine, inserted just before it.
_wsplit_counter = [0]


def _split_waits(nc, max_waits: int = 1):
    for func in nc.m.functions:
        for blk in func.blocks:
            insts = blk.instructions
            i = 0
            while i < len(insts):
                inst = insts[i]
                si = inst.sync_info
                waits = list(si.on_wait) if si is not None else []
                if len(waits) > max_waits:
                    keep = waits[-max_waits:]
                    hoist = waits[:-max_waits]
                    inst.sync_info = mybir.SyncInfo(
                        on_wait=keep, on_update=list(si.on_update)
                    )
                    new_insts = []
                    for w in hoist:
                        _wsplit_counter[0] += 1
                        ev = mybir.InstEventSemaphore(
                            name=f"WSPLIT-{_wsplit_counter[0]}", ins=[], outs=[]
                        )
                        ev.engine = inst.engine
                        ev.sync_info = mybir.SyncInfo(on_wait=[w], on_update=[])
                        new_insts.append(ev)
                    insts[i:i] = new_insts
                    i += len(new_insts)
                i += 1


# NTFF profiling shim: the axon .so supports NRT profiling but the antenv
# glue module is absent in this image; register it so trace=True works.
def _install_ntff_hook():
    if "antenv.axon_hooks" in sys.modules:
        return
    try:
        from trn_agent_boot.trn_boot import _ntff_profile_via_ctypes

        hook = _ntff_profile_via_ctypes("/opt/axon/libaxon_pjrt.so")
    except Exception:
        hook = None
    mod = types.ModuleType("antenv.axon_hooks")
    mod.get_axon_ntff_profile_hook = lambda: hook
    mod.set_axon_ntff_profile_hook = lambda h: None
    sys.modules["antenv.axon_hooks"] = mod


def _build():
    nc = bass.Bass("TRN2", target_bir_lowering=False, debug=False)
    qt = nc.dram_tensor("qt", [BPC, H, S], F32, kind="ExternalInput")
    kt = nc.dram_tensor("kt", [BPC, H, S], F32, kind="ExternalInput")
    v = nc.dram_tensor("v", [BPC, S, H], F32, kind="ExternalInput")
    out = nc.dram_tensor("out", [BPC, S, H], F32, kind="ExternalOutput")

    with tile.TileContext(nc) as tc, ExitStack() as ctx:
        consts = ctx.enter_context(tc.tile_pool(name="consts", bufs=1))
        raw = ctx.enter_context(tc.tile_pool(name="raw", bufs=4))
        qkv = ctx.enter_context(tc.tile_pool(name="qkv", bufs=2))
        et_pool = ctx.enter_context(tc.tile_pool(name="et", bufs=8))
        acc_pool = ctx.enter_context(tc.tile_pool(name="acc", bufs=5))
        sb_small = ctx.enter_context(tc.tile_pool(name="sb_small", bufs=2))
        outs = ctx.enter_context(tc.tile_pool(name="outs", bufs=2))
        ps_s = ctx.enter_context(tc.tile_pool(name="ps_s", bufs=2, space="PSUM"))
        ps_o = ctx.enter_context(tc.tile_pool(name="ps_o", bufs=1, space="PSUM"))
        ps_l = ctx.enter_context(tc.tile_pool(name="ps_l", bufs=1, space="PSUM"))

        ident = consts.tile([128, 128], F32)
        make_identity(nc, ident[:])
        # touch Exp first thing so the ACT table set loads under the DMAs
        warm = consts.tile([128, 2], F32)
        nc.vector.memset(warm[:], 0.0)
        nc.scalar.activation(warm[:], warm[:], Act.Exp)
        ones_f = consts.tile([128, 2], F32)
        nc.vector.memset(ones_f[:], 1.0)
        exp_bias = consts.tile([128, 1], F32)
        nc.vector.memset(exp_bias[:], -30.0)
        ones_r = consts.tile([128, 2], F32R)
        nc.vector.tensor_copy(ones_r[:], ones_f[:])
        # dummy matmul chain: keeps the PE busy during the initial DMAs so
        # the HAM clock-gate is at full rate when real matmuls arrive
        warm_z = consts.tile([128, 512], F32, tag="wz")
        nc.vector.memset(warm_z[:], 0.0)
        warm_r = consts.tile([128, 512], F32R)
        nc.vector.tensor_copy(warm_r[:], warm_z[:])
        ps_junk = ps_s.tile([128, 512], F32, tag="ps_s")
        for _ in range(10):
            nc.tensor.matmul(
                ps_junk[:], warm_r[:, 0:128], warm_r[:], start=True, stop=True
            )
        junk_sb = consts.tile([128, 2], F32, tag="wjunk")
        nc.vector.tensor_copy(junk_sb[:], ps_junk[:, 0:2])

        def emit_loads(b, fine):
            # load + round to f32r, chunked so compute starts early.
            # First batch uses finer leading chunks to cut the startup
            # serial path (DMA+cast before the first matmul).
            qr = qkv.tile([128, S], F32R, tag="qr")
            kr = qkv.tile([128, S], F32R, tag="kr")
            vr = qkv.tile([128, S], F32R, tag="vr")

            def load_k(lo, n):
                t = raw.tile([128, n], F32, tag="raw")
                nc.sync.dma_start(t[:], kt.ap()[b][:, bass.ds(lo, n)])
                nc.vector.tensor_copy(kr[:, bass.ds(lo, n)], t[:])

            def load_q(lo, n):
                t = raw.tile([128, n], F32, tag="raw")
                nc.sync.dma_start(t[:], qt.ap()[b][:, bass.ds(lo, n)])
                nc.vector.tensor_copy(qr[:, bass.ds(lo, n)], t[:])

            def load_v(lo, n):
                # v[b] rows [lo, lo+n) presented as [128p, (j 128h)]
                t = raw.tile([128, n], F32, tag="raw")
                v_chunk = bass.AP(
                    tensor=v,
                    offset=b * S * H + lo * H,
                    ap=[[H, 128], [128 * H, n // 128], [1, H]],
                )
                nc.sync.dma_start(t[:], v_chunk)
                nc.vector.tensor_copy(vr[:, bass.ds(lo, n)], t[:])

            if fine:
                load_k(0, 512)
                load_q(0, 256)
                load_v(0, 256)
                load_k(512, 512)
                load_q(256, 768)
                load_v(256, 768)
                load_k(1024, 1024)
                load_q(1024, 1024)
                load_v(1024, 1024)
            else:
                for hc in range(2):
                    load_k(hc * XH, XH)
                    load_q(hc * XH, XH)
                    load_v(hc * XH, XH)
            return qr, kr, vr

        qkv_b = {0: emit_loads(0, fine=True)}

        # Tail work for iteration (b, xh) is deferred into the NEXT
        # iteration's j-loop: the in-order PE queue then has the next
        # x-half's MM1s in front of the tail's small matmuls/transposes, so
        # the tail's DVE->PE dependency latency hides under real work.
        # part0 (emitted at loop end): evacuate po, then pl, on DVE.
        # part1 (next loop, it==3): l K=1 transposes + out transposes 0-3
        #   into one stolen ps_s slot; reciprocal; scale+copy 0-3.
        # part2 (next loop, it==9): out transposes 4-7; scale+copy; DMA out.
        def make_tail(b, xh, po, pl):
            st = {}

            def part0(l_first=False):
                l_sb = sb_small.tile([1, XH], F32R, tag="l_sb")
                if l_first:
                    # final tail: nothing follows, so put the l copy on the
                    # otherwise-idle ScalarE to run beside the DVE outu copy
                    nc.scalar.activation(l_sb[:], pl[0:1, :], Act.Identity)
                outu = outs.tile([128, XH], F32, tag="outu")
                nc.vector.tensor_copy(outu[:], po[:])
                if not l_first:
                    nc.vector.tensor_copy(l_sb[:], pl[0:1, :])
                out_sb = outs.tile([128, XH], F32, tag="out_sb")
                st.update(outu=outu, l_sb=l_sb, out_sb=out_sb)

            def part1():
                # one ps_s slot: cols 0-511 = transposes 0-3 (bank 0),
                # cols 512-527 = transposed l columns (bank 1).
                steal1 = ps_s.tile([128, 528], F32, tag="ps_s")
                for t in range(8):
                    nc.tensor.matmul(
                        steal1[:, 512 + 2 * t : 512 + 2 * t + 2],
                        st["l_sb"][0:1, bass.ts(t, 128)],
                        ones_r[0:1, 0:2],
                        start=True,
                        stop=True,
                    )
                for t in range(4):
                    nc.tensor.transpose(
                        steal1[:, bass.ts(t, 128)],
                        st["outu"][:, bass.ts(t, 128)],
                        ident[:],
                    )
                rl = sb_small.tile([128, 16], F32, tag="rl")
                nc.vector.reciprocal(rl[:], steal1[:, 512:528])
                for t in range(4):
                    nc.vector.tensor_scalar_mul(
                        st["out_sb"][:, bass.ts(t, 128)],
                        steal1[:, bass.ts(t, 128)],
                        rl[:, 2 * t : 2 * t + 1],
                    )
                st["rl"] = rl

            def part2():
                steal2 = ps_s.tile([128, 512], F32, tag="ps_s")
                for t in range(4):
                    nc.tensor.transpose(
                        steal2[:, bass.ts(t, 128)],
                        st["outu"][:, bass.ts(4 + t, 128)],
                        ident[:],
                    )
                for t in range(4):
                    nc.vector.tensor_scalar_mul(
                        st["out_sb"][:, bass.ts(4 + t, 128)],
                        steal2[:, bass.ts(t, 128)],
                        st["rl"][:, 2 * (4 + t) : 2 * (4 + t) + 1],
                    )
                # out[b] rows [xh*1024,...) as [128p, (8t 128h)], row=t*128+p
                out_view = bass.AP(
                    tensor=out,
                    offset=b * S * H + xh * 8 * 128 * H,
                    ap=[[H, 128], [128 * H, 8], [1, H]],
                )
                nc.sync.dma_start(out_view, st["out_sb"][:])

            return part0, part1, part2

        pending = None  # (part1, part2) of the previous (b, xh)

        # Software-pipelined emission: MM1(j) goes one iteration ahead of
        # MM2(j) so the in-order PE queue never waits on exp(j) with useful
        # MM1 work behind it, and the first two MM1/exp of the NEXT (b, xh)
        # are emitted inside the last two iterations of the current one so
        # the ACT exp chain never drains across loop boundaries.  The
        # softmax denominator is computed from DVE pair-sums
        # acc2(p) = eT(2p) + eT(2p+1), halving the PE ones-matmul streaming
        # cost, except the last pair which is read directly from eT so the
        # loop end does not wait on a DVE add.
        def emit_mm1_exp(qr, kr, xh, it, ets):
            pss = ps_s.tile([128, XH], F32, tag="ps_s")
            qj = qr[:, bass.ts(it, 128)]
            for c in range(2):
                nc.tensor.matmul(
                    pss[:, bass.ts(c, 512)],
                    qj,
                    kr[:, bass.ds(xh * XH + c * 512, 512)],
                    start=True,
                    stop=True,
                )
            et = et_pool.tile([128, XH], F32R, tag="et")
            ets[it] = et
            # bias -30 shifts the exp range: overflow now needs a score
            # > 118 instead of 88.7; the shift cancels exactly in the
            # softmax normalization (both numerator and l scale by e^-30)
            nc.scalar.activation(et[:], pss[:], Act.Exp, bias=exp_bias[:])

        seq = [(b, xh) for b in range(BPC) for xh in range(2)]
        heads = {}  # idx -> ets dict with pre-emitted iterations
        for idx, (b, xh) in enumerate(seq):
            qr, kr, vr = qkv_b[b]
            po = ps_o.tile([128, XH], F32)
            pl = ps_l.tile([2, XH], F32)
            ets = heads.pop(idx, {})
            accs = {}
            for it in range(NJ + 2):
                if it in (NJ, NJ + 1) and idx + 1 < len(seq):
                    # head of the next (b, xh): keep PE and ACT primed
                    nb, nxh = seq[idx + 1]
                    nqr, nkr, _ = qkv_b[nb]
                    h = heads.setdefault(idx + 1, {})
                    emit_mm1_exp(nqr, nkr, nxh, it - NJ, h)
                if it < NJ and it not in ets:
                    emit_mm1_exp(qr, kr, xh, it, ets)
                jj = it - 1
                if 0 <= jj < NJ:
                    vj = vr[:, bass.ts(jj, 128)]
                    for c in range(2):
                        nc.tensor.matmul(
                            po[:, bass.ts(c, 512)],
                            vj,
                            ets[jj][:, bass.ts(c, 512)],
                            start=(jj == 0),
                            stop=(jj == NJ - 1),
                        )
                # tail hook sits between MM2 and the pair-adds so the
                # tail's reciprocal/muls aren't queued behind a DVE add
                if pending is not None and it == 2:
                    pending[0]()
                if pending is not None and it == 8:
                    pending[1]()
                    pending = None
                if 0 <= jj < NJ:
                    if jj % 2 == 1 and jj < NJ - 2:
                        p = jj // 2
                        acc2 = acc_pool.tile([128, XH], F32R, tag="acc2")
                        accs[p] = acc2
                        nc.vector.tensor_add(
                            acc2[:], ets.pop(jj - 1)[:], ets.pop(jj)[:]
                        )
                    if jj == NJ - 1:
                        # last pair direct from eT: no DVE add on the
                        # loop-end critical path
                        last_two = [ets.pop(NJ - 2), ets.pop(NJ - 1)]
                        for srct in last_two:
                            for c in range(2):
                                nc.tensor.matmul(
                                    pl[0:2, bass.ts(c, 512)],
                                    ones_r[:],
                                    srct[:, bass.ts(c, 512)],
                                    start=False,
                                    stop=False,
                                )
                if it >= 5 and (it - 5) % 2 == 0 and (it - 5) // 2 < NJ // 2 - 1:
                    p = (it - 5) // 2
                    acc2 = accs.pop(p)
                    for c in range(2):
                        nc.tensor.matmul(
                            pl[0:2, bass.ts(c, 512)],
                            ones_r[:],
                            acc2[:, bass.ts(c, 512)],
                            start=(p == 0),
                            stop=(p == NJ // 2 - 2 and c == 1),
                        )
                if idx == 0 and it == 6 and BPC > 1:
                    # prefetch next batch; spread so the DVE casts don't
                    # pile up in one window
                    qkv_b[1] = emit_loads(1, fine=False)

            part0, part1, part2 = make_tail(b, xh, po, pl)
            part0(l_first=(idx == len(seq) - 1))
            pending = (part1, part2)

        # tail of the final (b, xh)
        pending[0]()
        pending[1]()

    _split_waits(nc)
    return nc


_NC_CACHE = None


def _get_nc():
    global _NC_CACHE
    if _NC_CACHE is None:
        _NC_CACHE = _build()
    return _NC_CACHE


def kernel(q: np.ndarray, k: np.ndarray, v: np.ndarray) -> np.ndarray:
    q = np.asarray(q, dtype=np.float32)
    k = np.asarray(k, dtype=np.float32)
    v = np.asarray(v, dtype=np.float32)
    qT = np.ascontiguousarray(q.transpose(0, 2, 1))  # [B, H, S]
    kT = np.ascontiguousarray(k.transpose(0, 2, 1))

    nc = _get_nc()
    in_maps = []
    for c in range(NCORES):
        sl = slice(BPC * c, BPC * (c + 1))
        in_maps.append(
            {
                "qt": np.ascontiguousarray(qT[sl]),
                "kt": np.ascontiguousarray(kT[sl]),
                "v": np.ascontiguousarray(v[sl]),
            }
        )

    trace = bool(int(os.environ.get("ATTN_KERNEL_TRACE", "0")))
    kwargs = {}
    if trace:
        _install_ntff_hook()
        kwargs["trace"] = True
        tmpdir = os.environ.get("ATTN_KERNEL_TRACE_DIR")
        if tmpdir:
            kwargs["tmpdir"] = tmpdir
    try:
        res = run_bass_kernel_spmd(
            nc, in_maps, core_ids=list(range(NCORES)), **kwargs
        )
    except Exception:
        # transient NRT/device hiccups have been observed once; retry
        res = run_bass_kernel_spmd(
            nc, in_maps, core_ids=list(range(NCORES)), **kwargs
        )
    if trace:
        kernel.last_results = res
    out = np.concatenate([res.results[c]["out"] for c in range(NCORES)], axis=0)
    return out.astype(np.float32)
